# revision 4
# baseline (speedup 1.0000x reference)
"""Trainium2 Bass kernel for nn_DataWindowLoss: mean(|box5x5(x) - box5x5(y)|).

Math: the 5x5 uniform box filter (padding=4) is linear and separable, so
    box(x) - box(y) = box(x - y) = A @ (x - y) @ A^T   (per image)
where A is the [516, 512] banded matrix with A[o, r] = 1 for o-4 <= r <= o.
Band entries are 1.0; the division by 25 happens on the host.

Per image (all contraction on the TensorEngine, fp16 operands):
  pass1: VT[w, o] = sum_r x[r, w]*A^T[r, o] - sum_r y[r, w]*A^T[r, o]
         (lhsT = x/y tiles, rhs = +/-1 banded constants, fp32 PSUM accum),
         drained PSUM->SBUF fp16 by the ScalarEngine (ACT).
  pass2: HT[c, o] = sum_w B[w, c] * VT[w, o] (lhsT = banded constants),
         each [128,516] PSUM chunk abs-sum-reduced by the VectorEngine
         into one fp32 column of an accumulator tile.

Schedule: the PE program software-pipelines images — pass2 chunks of image
i-1 are interleaved between pass1 m-tiles of image i, so PE never stalls
on the PSUM consumers (ACT drains pass1, DVE reduces pass2) and all
compute hides behind the input DMA stream (the kernel is HBM-bound:
16.8 MB of fp32 input per core; the fp32->fp16 cast rides the SWDGE DMA).

Sharding: pure data parallel - 8 images per core on 8 NeuronCores; each
core emits a [128, 40] fp32 partial-sum tile (5 pass2 chunks x 8 images);
the host reduces and normalizes.
"""

import sys

sys.path.insert(0, "/opt/trn_rl_repo")

import numpy as np

import concourse.mybir as mybir
import concourse.tile as tile
from concourse import bacc
from concourse.bass_utils import run_bass_kernel_spmd

N_CORES = 8
IMG_PER_CORE = 8
P = 128          # partitions
HW = 512         # image height/width
KT = 4           # r-tiles / w-tiles per image
OUT = 516        # output spatial size (512 + 2*4 - 5 + 1)
F16 = mybir.dt.float16
F32 = mybir.dt.float32


def _make_band_consts(nc, pool):
    """Banded +/-1 constants in fp16.

    bandP/bandN [128, 132]: band[p, j] = +/-1 iff p <= j <= p+4
    bandL/bandLn [128, 128]: corner[p, q] = +/-1 iff p - q >= 124
    """
    bandP = pool.tile([P, 132], F16)
    bandN = pool.tile([P, 132], F16)
    bandL = pool.tile([P, 128], F16)
    bandLn = pool.tile([P, 128], F16)
    for t, val in ((bandP, 1.0), (bandN, -1.0)):
        nc.gpsimd.memset(t, val)
        # keep iff j - p >= 0
        nc.gpsimd.affine_select(
            out=t, in_=t, compare_op=mybir.AluOpType.is_ge, fill=0.0,
            base=0, pattern=[[1, 132]], channel_multiplier=-1)
        # keep iff p + 4 - j >= 0
        nc.gpsimd.affine_select(
            out=t, in_=t, compare_op=mybir.AluOpType.is_ge, fill=0.0,
            base=4, pattern=[[-1, 132]], channel_multiplier=1)
    for t, val in ((bandL, 1.0), (bandLn, -1.0)):
        nc.gpsimd.memset(t, val)
        # keep iff p - q - 124 >= 0
        nc.gpsimd.affine_select(
            out=t, in_=t, compare_op=mybir.AluOpType.is_ge, fill=0.0,
            base=-124, pattern=[[-1, 128]], channel_multiplier=1)
    return bandP, bandN, bandL, bandLn


def _load_image(nc, x_sb, y_sb, x_dram, y_dram, i):
    """One full-image cast DMA per tensor (1MB fp32 HBM read -> fp16 SBUF).
    Measured best under 8-core HBM contention vs half-image or multi-image
    DMAs, HWDGE fp32 loads, and fp8 casts."""
    nc.gpsimd.dma_start(
        out=x_sb, in_=x_dram[i].rearrange("(k p) w -> p k w", p=P))
    nc.gpsimd.dma_start(
        out=y_sb, in_=y_dram[i].rearrange("(k p) w -> p k w", p=P))


def _emit_pass1_mtile(nc, consts, x_sb, y_sb, vt, m, vt_ps_pool):
    """One pass1 m-tile: 10 PE matmuls into PSUM + ACT drain to vt[:, m]."""
    bandP, bandN, bandL, bandLn = consts
    wb = slice(128 * m, 128 * (m + 1))
    vt_ps = vt_ps_pool.tile([P, OUT], F32)
    for src, band, bandc, isx in (
        (x_sb, bandP, bandL, True),
        (y_sb, bandN, bandLn, False),
    ):
        for k in range(KT):
            o0 = 128 * k
            n = 132 if k < 3 else 128
            nc.tensor.matmul(
                vt_ps[:, o0:o0 + n],
                lhsT=src[:, k, wb],
                rhs=band[:, 0:n],
                start=(isx and k == 0),
                stop=((not isx) and k == 3),
            )
        # o in [512, 516): contributions from rows 508..511
        nc.tensor.matmul(
            vt_ps[:, 512:516],
            lhsT=src[:, 3, wb],
            rhs=bandc[:, 0:4],
            start=isx,
            stop=not isx,
        )
    # drain PSUM fp32 -> SBUF fp16 on the ScalarEngine
    nc.scalar.copy(out=vt[:, m, :], in_=vt_ps)


def _emit_pass2_chunk(nc, consts, vt, m, jimg, h_ps_pool, acc):
    """One pass2 c-chunk: PE matmuls into PSUM + DVE abs-sum to acc col."""
    bandP, bandN, bandL, bandLn = consts
    h_ps = h_ps_pool.tile([P, OUT], F32)
    if m == 0:
        psl = slice(0, P)
        parts = [(bandP[:, 0:128], 0)]
    elif m <= 3:
        psl = slice(0, P)
        parts = [(bandP[:, 0:128], m), (bandL, m - 1)]
    else:
        psl = slice(0, 4)
        parts = [(bandL[:, 0:4], 3)]
    for j, (lhsT, wsrc) in enumerate(parts):
        first, last = j == 0, j == len(parts) - 1
        nc.tensor.matmul(
            h_ps[psl, 0:512], lhsT=lhsT, rhs=vt[:, wsrc, 0:512],
            start=first, stop=last)
        nc.tensor.matmul(
            h_ps[psl, 512:516], lhsT=lhsT, rhs=vt[:, wsrc, 512:516],
            start=first, stop=last)
    nc.vector.tensor_reduce(
        out=acc[psl, jimg * 5 + m:jimg * 5 + m + 1],
        in_=h_ps[psl, :],
        axis=mybir.AxisListType.X,
        op=mybir.AluOpType.add,
        apply_absolute_value=True,
    )


def build_module():
    nc = bacc.Bacc()
    x_dram = nc.dram_tensor("x", [IMG_PER_CORE, HW, HW], F32,
                            kind="ExternalInput")
    y_dram = nc.dram_tensor("y", [IMG_PER_CORE, HW, HW], F32,
                            kind="ExternalInput")
    out_dram = nc.dram_tensor("partials", [P, IMG_PER_CORE * 5], F32,
                              kind="ExternalOutput")

    with tile.TileContext(nc) as tc:
        with (
            tc.tile_pool(name="consts", bufs=1) as consts_pool,
            tc.tile_pool(name="xin", bufs=8) as xpool,
            tc.tile_pool(name="yin", bufs=8) as ypool,
            tc.tile_pool(name="vt", bufs=4) as vtpool,
            tc.tile_pool(name="accp", bufs=1) as accpool,
            tc.tile_pool(name="vtps", bufs=2, space="PSUM") as vt_ps_pool,
            tc.tile_pool(name="hps", bufs=2, space="PSUM") as h_ps_pool,
        ):
            # Image 0's loads trace before the const-building: SWDGE emission
            # shares the Pool (gpsimd) engine with memset/affine_select, so
            # this starts HBM traffic earlier.
            x0_sb = xpool.tile([P, KT, HW], F16, name="x_sb")
            y0_sb = ypool.tile([P, KT, HW], F16, name="y_sb")
            _load_image(nc, x0_sb, y0_sb, x_dram, y_dram, 0)

            consts = _make_band_consts(nc, consts_pool)
            # 5 abs-sum columns per image, fp32
            acc = accpool.tile([P, IMG_PER_CORE * 5], F32)
            nc.vector.memset(acc, 0.0)

            prev = None  # (img_idx, vt tile) pending pass2
            for i in range(IMG_PER_CORE):
                if i == 0:
                    x_sb, y_sb = x0_sb, y0_sb
                else:
                    x_sb = xpool.tile([P, KT, HW], F16, name="x_sb")
                    y_sb = ypool.tile([P, KT, HW], F16, name="y_sb")
                    _load_image(nc, x_sb, y_sb, x_dram, y_dram, i)

                # Software pipeline on the in-order PE queue: pass2 chunks of
                # image i-1 interleave between pass1 m-tiles of image i, so
                # PE work continues while ACT/DVE drain the previous PSUM
                # tiles (2-buf PSUM pools).
                vt = vtpool.tile([P, KT, OUT], F16)
                for m in range(KT):
                    _emit_pass1_mtile(nc, consts, x_sb, y_sb, vt, m,
                                      vt_ps_pool)
                    if prev is not None:
                        _emit_pass2_chunk(nc, consts, prev[1], m, prev[0],
                                          h_ps_pool, acc)
                if prev is not None:
                    _emit_pass2_chunk(nc, consts, prev[1], 4, prev[0],
                                      h_ps_pool, acc)
                prev = (i, vt)

            for m in range(5):
                _emit_pass2_chunk(nc, consts, prev[1], m, prev[0],
                                  h_ps_pool, acc)
            nc.sync.dma_start(out=out_dram[:], in_=acc)

    nc.finalize()
    return nc


_NC_CACHE = None


def kernel(x: np.ndarray, y: np.ndarray) -> np.ndarray:
    global _NC_CACHE
    if _NC_CACHE is None:
        _NC_CACHE = build_module()
    nc = _NC_CACHE

    x = np.ascontiguousarray(np.asarray(x, dtype=np.float32).reshape(64, HW, HW))
    y = np.ascontiguousarray(np.asarray(y, dtype=np.float32).reshape(64, HW, HW))

    in_maps = [
        {
            "x": x[IMG_PER_CORE * c:IMG_PER_CORE * (c + 1)],
            "y": y[IMG_PER_CORE * c:IMG_PER_CORE * (c + 1)],
        }
        for c in range(N_CORES)
    ]
    res = run_bass_kernel_spmd(nc, in_maps, core_ids=list(range(N_CORES)))
    total = np.float64(0.0)
    for r in res.results:
        total += r["partials"].astype(np.float64).sum()
    mean = total / (25.0 * 64 * OUT * OUT)
    return np.float32(mean)
